# revision 2
# baseline (speedup 1.0000x reference)
"""Multi-head causal self-attention (B=2, S=2048, D=2048, H=16) on 8 TRN2 cores.

Sharding: data parallel on batch (2) x tensor parallel on head groups (4 heads
per core). Each core computes QKV projections for its 512 q/k/v channels, the
causal attention for its 4 heads, and a partial output projection against its
512 columns of Wo. The host sums the 4 partials per batch and adds bo.

All matmul operands are fp16; softmax statistics stay fp32. Scores are in
[k, q] orientation so exp'd tiles feed the PV matmul directly; row sums come
from an all-ones stationary matmul and normalization happens on the PSUM->SBUF
copy via a fast approximate reciprocal.

The whole kernel is emitted as four interleaved rounds (one per 512-column
sequence group): QK projections, V projection, attention for the q-group, and
the output-projection rows it unlocks. The Tile scheduler overlaps rounds, so
projection matmuls fill the PE bubbles left by the scores->exp->PV dependency
chain. PSUM tags are sized to exactly 8 banks so no phase falsely serializes.
"""

import math
from contextlib import ExitStack

import numpy as np

import concourse.bass as bass
import concourse.tile as tile
from concourse import bacc, mybir
from concourse.bass_utils import run_bass_kernel_spmd

B, S, D, H, HD = 2, 2048, 2048, 16, 128
N_CORES = 8
HPC = 4          # heads per core
HJ = HPC * HD    # 512 projection channels per core
SG = 512         # column-group width for matmuls
ND = D // 128    # 16 contraction tiles over model dim
NS = S // 128    # 16 tiles over sequence
NG = S // SG     # 4 column groups over sequence

F32 = mybir.dt.float32
F16 = mybir.dt.float16
ADD = mybir.AluOpType.add
MUL = mybir.AluOpType.mult
EXP = mybir.ActivationFunctionType.Exp

last_exec_time_ns = None


def _build():
    nc = bacc.Bacc("TRN2", target_bir_lowering=False, debug=False)

    xt = nc.dram_tensor("xt", [D, S], F16, kind="ExternalInput").ap()
    wq = nc.dram_tensor("wq", [D, HJ], F16, kind="ExternalInput").ap()
    wk = nc.dram_tensor("wk", [D, HJ], F16, kind="ExternalInput").ap()
    wv = nc.dram_tensor("wv", [D, HJ], F16, kind="ExternalInput").ap()
    wo = nc.dram_tensor("wo", [HJ, D], F16, kind="ExternalInput").ap()
    bq = nc.dram_tensor("bq", [HJ, 1], F32, kind="ExternalInput").ap()
    bk = nc.dram_tensor("bk", [HJ, 1], F32, kind="ExternalInput").ap()
    bv = nc.dram_tensor("bv", [1, HJ], F16, kind="ExternalInput").ap()
    mask = nc.dram_tensor("mask", [128, 128], F32, kind="ExternalInput").ap()
    ones = nc.dram_tensor("ones", [1, 128], F16, kind="ExternalInput").ap()
    out = nc.dram_tensor("out", [S, D], F16, kind="ExternalOutput").ap()

    with tile.TileContext(nc) as tc, ExitStack() as es:
        cpool = es.enter_context(tc.tile_pool(name="const", bufs=1))
        mask_sb = cpool.tile([128, 128], F32, name="mask_sb", tag="mask")
        nc.sync.dma_start(mask_sb[:], mask[:])
        ones_sb = cpool.tile([1, 128], F16, name="ones_sb", tag="ones")
        nc.sync.dma_start(ones_sb[:], ones[:])
        bv_sb = cpool.tile([1, HJ], F16, name="bv_sb", tag="bv")
        nc.sync.dma_start(bv_sb[:], bv[:])
        onesm_sb = cpool.tile([128, 128], F16, name="onesm_sb", tag="onesm")
        nc.gpsimd.memset(onesm_sb[:], 1.0)
        bq_sb = []
        bk_sb = []
        for i in range(HPC):
            t = cpool.tile([128, 1], F32, name=f"bq{i}", tag=f"bq{i}")
            nc.sync.dma_start(t[:], bq[i * 128:(i + 1) * 128, :])
            bq_sb.append(t)
            t = cpool.tile([128, 1], F32, name=f"bk{i}", tag=f"bk{i}")
            nc.sync.dma_start(t[:], bk[i * 128:(i + 1) * 128, :])
            bk_sb.append(t)

        # weights live in SBUF for the whole kernel, one big DMA each
        wpool = es.enter_context(tc.tile_pool(name="wts", bufs=1))
        wq_sb = wpool.tile([128, ND, HJ], F16, name="wq_sb", tag="wq")
        nc.sync.dma_start(wq_sb[:], wq.rearrange("(d p) h -> p d h", p=128))
        wk_sb = wpool.tile([128, ND, HJ], F16, name="wk_sb", tag="wk")
        nc.sync.dma_start(wk_sb[:], wk.rearrange("(d p) h -> p d h", p=128))
        wv_sb = wpool.tile([128, ND, HJ], F16, name="wv_sb", tag="wv")
        nc.sync.dma_start(wv_sb[:], wv.rearrange("(d p) h -> p d h", p=128))
        wo_sb = wpool.tile([128, HPC, D], F16, name="wo_sb", tag="wo")
        nc.sync.dma_start(wo_sb[:], wo.rearrange("(t p) e -> p t e", p=128))

        rpool = es.enter_context(tc.tile_pool(name="res", bufs=1))
        qT = [rpool.tile([128, S], F16, name=f"qT{i}", tag=f"qT{i}")
              for i in range(HPC)]
        kT = [rpool.tile([128, S], F16, name=f"kT{i}", tag=f"kT{i}")
              for i in range(HPC)]
        vsb = [rpool.tile([128, HJ], F16, name=f"v{j}", tag=f"v{j}")
               for j in range(NS)]
        attn = [rpool.tile([128, S], F16, name=f"at{h}", tag=f"at{h}")
                for h in range(HPC)]

        xpool = es.enter_context(tc.tile_pool(name="xts", bufs=2))
        etpool = es.enter_context(tc.tile_pool(name="et", bufs=8))
        rrpool = es.enter_context(tc.tile_pool(name="rr", bufs=2))
        opool = es.enter_context(tc.tile_pool(name="ost", bufs=3))
        # exactly 8 PSUM banks: qk 2, v 1, sc 2, pv 1, sm 1, p3 1
        ps_qk = es.enter_context(tc.tile_pool(name="ps_qk", bufs=2, space="PSUM"))
        ps_v = es.enter_context(tc.tile_pool(name="ps_v", bufs=1, space="PSUM"))
        ps_sc = es.enter_context(tc.tile_pool(name="ps_sc", bufs=2, space="PSUM"))
        ps_pv = es.enter_context(tc.tile_pool(name="ps_pv", bufs=1, space="PSUM"))
        ps_sm = es.enter_context(tc.tile_pool(name="ps_sm", bufs=1, space="PSUM"))
        ps_p3 = es.enter_context(tc.tile_pool(name="ps_p3", bufs=1, space="PSUM"))

        xr = xt.rearrange("(d p) s -> p d s", p=128)

        for sg in range(NG):
            x_sb = xpool.tile([128, ND, SG], F16, name="x_sb", tag="xt")
            nc.sync.dma_start(x_sb[:], xr[:, :, sg * SG:(sg + 1) * SG])

            # q/k projections, one (which, head) output block at a time so a
            # block's 16 contraction matmuls run back to back into one bank
            for wsb, bias, dst in ((wq_sb, bq_sb, qT), (wk_sb, bk_sb, kT)):
                for i in range(HPC):
                    ps = ps_qk.tile([128, SG], F32, name="ps_qk_t", tag="qk")
                    for d in range(ND):
                        nc.tensor.matmul(
                            ps[:], lhsT=wsb[:, d, i * 128:(i + 1) * 128],
                            rhs=x_sb[:, d, :],
                            start=(d == 0), stop=(d == ND - 1))
                    nc.vector.tensor_scalar_add(
                        dst[i][:, sg * SG:(sg + 1) * SG], ps[:], bias[i][:])

            # v projection in natural [s, hj] layout; bias via rank-1 matmul
            for ss in range(4):
                ps = ps_v.tile([128, HJ], F32, name="ps_v_t", tag="v")
                for d in range(ND):
                    nc.tensor.matmul(
                        ps[:], lhsT=x_sb[:, d, ss * 128:(ss + 1) * 128],
                        rhs=wv_sb[:, d, :],
                        start=(d == 0), stop=False)
                nc.tensor.matmul(
                    ps[:], lhsT=ones_sb[:], rhs=bv_sb[:],
                    start=False, stop=True)
                nc.vector.tensor_copy(vsb[sg * 4 + ss][:], ps[:])

            # attention for q-group g = sg (needs only k/v tiles <= this group)
            g = sg
            nkt = 4 * g + 4
            for h in range(HPC):
                po = ps_pv.tile([128, SG], F32, name="po_t", tag="pv")
                sm = ps_sm.tile([128, SG], F32, name="sm_t", tag="sm")
                for kt in range(nkt):
                    jlo = max(0, kt - 4 * g)
                    qoff = jlo * 128
                    w = SG - qoff
                    psc = ps_sc.tile([128, SG], F32, name="psc_t", tag="sc")
                    nc.tensor.matmul(
                        psc[:, :w],
                        lhsT=kT[h][:, kt * 128:(kt + 1) * 128],
                        rhs=qT[h][:, g * SG + qoff:(g + 1) * SG],
                        start=True, stop=True)
                    if kt >= 4 * g:
                        # diagonal block is this tile's first 128 cols
                        nc.vector.tensor_tensor(
                            psc[:, 0:128], psc[:, 0:128], mask_sb[:], op=ADD)
                    et = etpool.tile([128, SG], F16, name="et_t", tag="et")
                    nc.scalar.activation(et[:, :w], psc[:, :w], EXP)
                    nc.tensor.matmul(
                        po[:, qoff:],
                        lhsT=vsb[kt][:, h * 128:(h + 1) * 128],
                        rhs=et[:, :w],
                        start=(kt == 0), stop=(kt == nkt - 1))
                    nc.tensor.matmul(
                        sm[:, qoff:], lhsT=onesm_sb[:], rhs=et[:, :w],
                        start=(kt == 0), stop=(kt == nkt - 1))
                rr = rrpool.tile([128, SG], F32, name="rr_t", tag="rr")
                nc.vector.reciprocal_approx_fast(rr[:], sm[:])
                nc.vector.tensor_tensor(
                    attn[h][:, g * SG:(g + 1) * SG], po[:], rr[:], op=MUL)

            # output-projection rows unlocked by this group: st = 4g..4g+3
            for st in range(4 * g, 4 * g + 4):
                for dg in range(NG):
                    po3 = ps_p3.tile([128, SG], F32, name="po3_t", tag="p3")
                    for h in range(HPC):
                        nc.tensor.matmul(
                            po3[:],
                            lhsT=attn[h][:, st * 128:(st + 1) * 128],
                            rhs=wo_sb[:, h, dg * SG:(dg + 1) * SG],
                            start=(h == 0), stop=(h == HPC - 1))
                    ot = opool.tile([128, SG], F16, name="ot_t", tag="ost")
                    nc.vector.tensor_copy(ot[:], po3[:])
                    nc.gpsimd.dma_start(
                        out[st * 128:(st + 1) * 128,
                            dg * SG:(dg + 1) * SG], ot[:])

    nc.finalize()
    return nc


_NC_CACHE = []


def kernel(hidden_states, Wq, bq, Wk, bk, Wv, bv, Wo, bo, **_unused):
    global last_exec_time_ns

    hidden_states = np.asarray(hidden_states, dtype=np.float32)
    Wq = np.asarray(Wq, dtype=np.float32)
    Wk = np.asarray(Wk, dtype=np.float32)
    Wv = np.asarray(Wv, dtype=np.float32)
    Wo = np.asarray(Wo, dtype=np.float32)
    bq = np.asarray(bq, dtype=np.float32)
    bk = np.asarray(bk, dtype=np.float32)
    bv = np.asarray(bv, dtype=np.float32)
    bo = np.asarray(bo, dtype=np.float32)

    if not _NC_CACHE:
        _NC_CACHE.append(_build())
    nc = _NC_CACHE[0]

    scale = 1.0 / math.sqrt(HD)
    q_idx = np.arange(128)[:, None]
    k_idx = np.arange(128)[None, :]
    # [k, q] orientation: keep k <= q
    mask = np.where(k_idx.T <= q_idx.T, 0.0, -50.0).astype(np.float32)
    ones = np.ones((1, 128), np.float16)

    xts = [np.ascontiguousarray(hidden_states[b].T).astype(np.float16)
           for b in range(B)]
    in_maps = []
    for c in range(N_CORES):
        b, hg = divmod(c, HPC)
        sl = slice(hg * HJ, (hg + 1) * HJ)
        in_maps.append({
            "xt": xts[b],
            "wq": np.ascontiguousarray((Wq[sl] * scale).T).astype(np.float16),
            "wk": np.ascontiguousarray(Wk[sl].T).astype(np.float16),
            "wv": np.ascontiguousarray(Wv[sl].T).astype(np.float16),
            "wo": np.ascontiguousarray(Wo[:, sl].T).astype(np.float16),
            "bq": (bq[sl] * scale).reshape(HJ, 1).copy(),
            "bk": bk[sl].reshape(HJ, 1).copy(),
            "bv": bv[sl].reshape(1, HJ).astype(np.float16),
            "mask": mask,
            "ones": ones,
        })

    res = run_bass_kernel_spmd(nc, in_maps, core_ids=list(range(N_CORES)))
    last_exec_time_ns = res.exec_time_ns

    outp = np.empty((B, S, D), np.float32)
    for b in range(B):
        acc = res.results[b * HPC]["out"].astype(np.float32)
        for c in range(b * HPC + 1, (b + 1) * HPC):
            acc = acc + res.results[c]["out"].astype(np.float32)
        outp[b] = acc + bo[None, :]
    return outp


# revision 7
# speedup vs baseline: 1.0664x; 1.0664x over previous
"""Multi-head causal self-attention (B=2, S=2048, D=2048, H=16) on 8 TRN2 cores.

Sharding: data parallel on batch (2) x tensor parallel on head groups (4 heads
per core). Each core computes QKV projections for its 512 q/k/v channels, the
causal attention for its 4 heads, and a partial output projection against its
512 columns of Wo. The host sums the 4 partials per batch and adds bo.

All matmul operands are fp16; softmax statistics stay fp32. Scores are in
[k, q] orientation so exp'd tiles feed the PV matmul directly; row sums come
from an all-ones stationary matmul and normalization happens on the PSUM->SBUF
copy via a fast approximate reciprocal.

The whole kernel is emitted as four interleaved rounds (one per 512-column
sequence group): QK projections, V projection, attention for the q-group, and
the output-projection rows it unlocks. The Tile scheduler overlaps rounds, so
projection matmuls fill the PE bubbles left by the scores->exp->PV dependency
chain. PSUM tags are sized to exactly 8 banks so no phase falsely serializes.
"""

import math
from contextlib import ExitStack

import numpy as np

import concourse.bass as bass
import concourse.tile as tile
from concourse import bacc, mybir
from concourse.bass_utils import run_bass_kernel_spmd

B, S, D, H, HD = 2, 2048, 2048, 16, 128
N_CORES = 8
HPC = 4          # heads per core
HJ = HPC * HD    # 512 projection channels per core
SG = 512         # column-group width for matmuls
ND = D // 128    # 16 contraction tiles over model dim
NS = S // 128    # 16 tiles over sequence
NG = S // SG     # 4 column groups over sequence

F32 = mybir.dt.float32
F16 = mybir.dt.float16
ADD = mybir.AluOpType.add
MUL = mybir.AluOpType.mult
EXP = mybir.ActivationFunctionType.Exp

last_exec_time_ns = None


def _build():
    nc = bacc.Bacc("TRN2", target_bir_lowering=False, debug=False)

    xt = nc.dram_tensor("xt", [D, S], F16, kind="ExternalInput").ap()
    wq = nc.dram_tensor("wq", [D, HJ], F16, kind="ExternalInput").ap()
    wk = nc.dram_tensor("wk", [D, HJ], F16, kind="ExternalInput").ap()
    wv = nc.dram_tensor("wv", [D, HJ], F16, kind="ExternalInput").ap()
    wo = nc.dram_tensor("wo", [HJ, D], F16, kind="ExternalInput").ap()
    bq = nc.dram_tensor("bq", [HJ, 1], F32, kind="ExternalInput").ap()
    bk = nc.dram_tensor("bk", [HJ, 1], F32, kind="ExternalInput").ap()
    bv = nc.dram_tensor("bv", [1, HJ], F16, kind="ExternalInput").ap()
    mask = nc.dram_tensor("mask", [128, 128], F32, kind="ExternalInput").ap()
    ones = nc.dram_tensor("ones", [1, 128], F16, kind="ExternalInput").ap()
    out = nc.dram_tensor("out", [S, D], F16, kind="ExternalOutput").ap()

    with tile.TileContext(nc) as tc, ExitStack() as es:
        cpool = es.enter_context(tc.tile_pool(name="const", bufs=1))
        mask_sb = cpool.tile([128, 128], F32, name="mask_sb", tag="mask")
        nc.sync.dma_start(mask_sb[:], mask[:])
        ones_sb = cpool.tile([1, 128], F16, name="ones_sb", tag="ones")
        nc.sync.dma_start(ones_sb[:], ones[:])
        bv_sb = cpool.tile([1, HJ], F16, name="bv_sb", tag="bv")
        nc.sync.dma_start(bv_sb[:], bv[:])
        onesm_sb = cpool.tile([128, 128], F16, name="onesm_sb", tag="onesm")
        nc.gpsimd.memset(onesm_sb[:], 1.0)
        bq_sb = []
        bk_sb = []
        for i in range(HPC):
            t = cpool.tile([128, 1], F32, name=f"bq{i}", tag=f"bq{i}")
            nc.sync.dma_start(t[:], bq[i * 128:(i + 1) * 128, :])
            bq_sb.append(t)
            t = cpool.tile([128, 1], F32, name=f"bk{i}", tag=f"bk{i}")
            nc.sync.dma_start(t[:], bk[i * 128:(i + 1) * 128, :])
            bk_sb.append(t)

        # weights live in SBUF for the whole kernel. Startup loads are the
        # critical path to the first matmul, so x(sg=0) and each weight are
        # split between the two HWDGE queues (sync=SP, scalar=Act) to double
        # transfer parallelism, ordered x0, wq, wk, wv, rest.
        wpool = es.enter_context(tc.tile_pool(name="wts", bufs=1))
        wq_sb = wpool.tile([128, ND, HJ], F16, name="wq_sb", tag="wq")
        wk_sb = wpool.tile([128, ND, HJ], F16, name="wk_sb", tag="wk")
        wv_sb = wpool.tile([128, ND, HJ], F16, name="wv_sb", tag="wv")
        wo_sb = wpool.tile([128, HPC, D], F16, name="wo_sb", tag="wo")
        xpool = es.enter_context(tc.tile_pool(name="xts", bufs=2))
        xr = xt.rearrange("(d p) s -> p d s", p=128)
        HND = ND // 2
        x0_sb = xpool.tile([128, ND, SG], F16, name="x_sb", tag="xt")
        nc.sync.dma_start(x0_sb[:, 0:HND, :], xr[:, 0:HND, 0:SG])
        nc.scalar.dma_start(x0_sb[:, HND:ND, :], xr[:, HND:ND, 0:SG])
        for dst, src in ((wq_sb, wq), (wk_sb, wk), (wv_sb, wv)):
            r = src.rearrange("(d p) h -> p d h", p=128)
            nc.sync.dma_start(dst[:, 0:HND, :], r[:, 0:HND, :])
            nc.scalar.dma_start(dst[:, HND:ND, :], r[:, HND:ND, :])
        wor = wo.rearrange("(t p) e -> p t e", p=128)
        nc.sync.dma_start(wo_sb[:, 0:2, :], wor[:, 0:2, :])
        nc.scalar.dma_start(wo_sb[:, 2:4, :], wor[:, 2:4, :])

        rpool = es.enter_context(tc.tile_pool(name="res", bufs=1))
        qT = [rpool.tile([128, S], F16, name=f"qT{i}", tag=f"qT{i}")
              for i in range(HPC)]
        kT = [rpool.tile([128, S], F16, name=f"kT{i}", tag=f"kT{i}")
              for i in range(HPC)]
        vsb = [rpool.tile([128, HJ], F16, name=f"v{j}", tag=f"v{j}")
               for j in range(NS)]
        attn = [rpool.tile([128, S], F16, name=f"at{h}", tag=f"at{h}")
                for h in range(HPC)]

        etpool = es.enter_context(tc.tile_pool(name="et", bufs=8))
        rrpool = es.enter_context(tc.tile_pool(name="rr", bufs=2))
        opool = es.enter_context(tc.tile_pool(name="ost", bufs=3))
        # exactly 8 PSUM banks: qk 2, v 1, sc 2, pv 1, sm 1, p3 1
        ps_qk = es.enter_context(tc.tile_pool(name="ps_qk", bufs=2, space="PSUM"))
        ps_v = es.enter_context(tc.tile_pool(name="ps_v", bufs=1, space="PSUM"))
        ps_sc = es.enter_context(tc.tile_pool(name="ps_sc", bufs=2, space="PSUM"))
        ps_pv = es.enter_context(tc.tile_pool(name="ps_pv", bufs=1, space="PSUM"))
        ps_sm = es.enter_context(tc.tile_pool(name="ps_sm", bufs=1, space="PSUM"))
        ps_p3 = es.enter_context(tc.tile_pool(name="ps_p3", bufs=1, space="PSUM"))

        for sg in range(NG):
            if sg == 0:
                x_sb = x0_sb
            else:
                x_sb = xpool.tile([128, ND, SG], F16, name="x_sb", tag="xt")
                nc.sync.dma_start(x_sb[:], xr[:, :, sg * SG:(sg + 1) * SG])

            # q/k projections, one (which, head) output block at a time so a
            # block's 16 contraction matmuls run back to back into one bank
            for wsb, bias, dst in ((wq_sb, bq_sb, qT), (wk_sb, bk_sb, kT)):
                for i in range(HPC):
                    ps = ps_qk.tile([128, SG], F32, name="ps_qk_t", tag="qk")
                    for d in range(ND):
                        nc.tensor.matmul(
                            ps[:], lhsT=wsb[:, d, i * 128:(i + 1) * 128],
                            rhs=x_sb[:, d, :],
                            start=(d == 0), stop=(d == ND - 1))
                    nc.vector.tensor_scalar_add(
                        dst[i][:, sg * SG:(sg + 1) * SG], ps[:], bias[i][:])

            # v projection in natural [s, hj] layout; bias via rank-1 matmul
            for ss in range(4):
                ps = ps_v.tile([128, HJ], F32, name="ps_v_t", tag="v")
                for d in range(ND):
                    nc.tensor.matmul(
                        ps[:], lhsT=x_sb[:, d, ss * 128:(ss + 1) * 128],
                        rhs=wv_sb[:, d, :],
                        start=(d == 0), stop=False)
                nc.tensor.matmul(
                    ps[:], lhsT=ones_sb[:], rhs=bv_sb[:],
                    start=False, stop=True)
                nc.vector.tensor_copy(vsb[sg * 4 + ss][:], ps[:])

            # attention for q-group g = sg (needs only k/v tiles <= this group)
            g = sg
            nkt = 4 * g + 4
            for h in range(HPC):
                po = ps_pv.tile([128, SG], F32, name="po_t", tag="pv")
                sm = ps_sm.tile([128, SG], F32, name="sm_t", tag="sm")
                for kt in range(nkt):
                    jlo = max(0, kt - 4 * g)
                    qoff = jlo * 128
                    w = SG - qoff
                    psc = ps_sc.tile([128, SG], F32, name="psc_t", tag="sc")
                    nc.tensor.matmul(
                        psc[:, :w],
                        lhsT=kT[h][:, kt * 128:(kt + 1) * 128],
                        rhs=qT[h][:, g * SG + qoff:(g + 1) * SG],
                        start=True, stop=True)
                    if kt >= 4 * g:
                        # diagonal block is this tile's first 128 cols
                        nc.vector.tensor_tensor(
                            psc[:, 0:128], psc[:, 0:128], mask_sb[:], op=ADD)
                    et = etpool.tile([128, SG], F16, name="et_t", tag="et")
                    nc.scalar.activation(et[:, :w], psc[:, :w], EXP)
                    nc.tensor.matmul(
                        po[:, qoff:],
                        lhsT=vsb[kt][:, h * 128:(h + 1) * 128],
                        rhs=et[:, :w],
                        start=(kt == 0), stop=(kt == nkt - 1))
                    nc.tensor.matmul(
                        sm[:, qoff:], lhsT=onesm_sb[:], rhs=et[:, :w],
                        start=(kt == 0), stop=(kt == nkt - 1))
                rr = rrpool.tile([128, SG], F32, name="rr_t", tag="rr")
                nc.vector.reciprocal_approx_fast(rr[:], sm[:])
                nc.vector.tensor_tensor(
                    attn[h][:, g * SG:(g + 1) * SG], po[:], rr[:], op=MUL)

            # output-projection rows unlocked by this group: st = 4g..4g+3.
            # In the last group the projection pipeline is drained, so rotate
            # po3 through the idle qk banks too to avoid bank-reuse stalls;
            # the PSUM->SBUF copy runs on the (then idle) scalar engine.
            for bi, (st, dg) in enumerate(
                    (st, dg)
                    for st in range(4 * g, 4 * g + 4) for dg in range(NG)):
                if g == NG - 1 and bi % 2 == 1:
                    po3 = ps_qk.tile([128, SG], F32, name="po3_t", tag="qk")
                else:
                    po3 = ps_p3.tile([128, SG], F32, name="po3_t", tag="p3")
                for h in range(HPC):
                    nc.tensor.matmul(
                        po3[:],
                        lhsT=attn[h][:, st * 128:(st + 1) * 128],
                        rhs=wo_sb[:, h, dg * SG:(dg + 1) * SG],
                        start=(h == 0), stop=(h == HPC - 1))
                ot = opool.tile([128, SG], F16, name="ot_t", tag="ost")
                nc.scalar.copy(ot[:], po3[:])
                nc.gpsimd.dma_start(
                    out[st * 128:(st + 1) * 128,
                        dg * SG:(dg + 1) * SG], ot[:])

    nc.finalize()
    return nc


_NC_CACHE = []


def kernel(hidden_states, Wq, bq, Wk, bk, Wv, bv, Wo, bo, **_unused):
    global last_exec_time_ns

    hidden_states = np.asarray(hidden_states, dtype=np.float32)
    Wq = np.asarray(Wq, dtype=np.float32)
    Wk = np.asarray(Wk, dtype=np.float32)
    Wv = np.asarray(Wv, dtype=np.float32)
    Wo = np.asarray(Wo, dtype=np.float32)
    bq = np.asarray(bq, dtype=np.float32)
    bk = np.asarray(bk, dtype=np.float32)
    bv = np.asarray(bv, dtype=np.float32)
    bo = np.asarray(bo, dtype=np.float32)

    if not _NC_CACHE:
        _NC_CACHE.append(_build())
    nc = _NC_CACHE[0]

    scale = 1.0 / math.sqrt(HD)
    q_idx = np.arange(128)[:, None]
    k_idx = np.arange(128)[None, :]
    # [k, q] orientation: keep k <= q
    mask = np.where(k_idx.T <= q_idx.T, 0.0, -50.0).astype(np.float32)
    ones = np.ones((1, 128), np.float16)

    xts = [np.ascontiguousarray(hidden_states[b].T).astype(np.float16)
           for b in range(B)]
    in_maps = []
    for c in range(N_CORES):
        b, hg = divmod(c, HPC)
        sl = slice(hg * HJ, (hg + 1) * HJ)
        in_maps.append({
            "xt": xts[b],
            "wq": np.ascontiguousarray((Wq[sl] * scale).T).astype(np.float16),
            "wk": np.ascontiguousarray(Wk[sl].T).astype(np.float16),
            "wv": np.ascontiguousarray(Wv[sl].T).astype(np.float16),
            "wo": np.ascontiguousarray(Wo[:, sl].T).astype(np.float16),
            "bq": (bq[sl] * scale).reshape(HJ, 1).copy(),
            "bk": bk[sl].reshape(HJ, 1).copy(),
            "bv": bv[sl].reshape(1, HJ).astype(np.float16),
            "mask": mask,
            "ones": ones,
        })

    res = run_bass_kernel_spmd(nc, in_maps, core_ids=list(range(N_CORES)))
    last_exec_time_ns = res.exec_time_ns

    outp = np.empty((B, S, D), np.float32)
    for b in range(B):
        acc = res.results[b * HPC]["out"].astype(np.float32)
        for c in range(b * HPC + 1, (b + 1) * HPC):
            acc = acc + res.results[c]["out"].astype(np.float32)
        outp[b] = acc + bo[None, :]
    return outp
